# revision 7
# baseline (speedup 1.0000x reference)
"""2D DCT-II (4096x4096, f32) on 8 Trainium2 NeuronCores.

out = Cm @ x @ Cn^T with Cm[u,i] = cos(pi*(2i+1)*u/(2M)) — mathematically
identical to the reference's Makhoul-FFT formulation.

Pass 1 contracts over x's row index i (K=2048 after the host row-fold).
Pass 2 contracts over x's column index c. Every c-axis fold that pass 2
needs (g2/h2 level-1, gg2/hh2 level-2, and a level-3 fold of the ee
branch) distributes over pass 1's matmul and is therefore applied by the
HOST to the pass-1 input columns: A^T[c,t] +/- A^T[C-1-c,t] =
sum_i (g[i,c] +/- g[i,C-1-c]) cmt[i,t]. The device never reverses or
folds anything — pass 1 directly emits the folded operands pass 2 needs:
  xr cols 0:512    -> gg3p (feeds ee_e, K=512, out v = 8r)
  xr cols 512:1024 -> gg3m (feeds ee_o, K=512, out v = 8r+4)
  xr cols 1024:2048-> hh2  (feeds eo,   K=1024, out v = 4r+2)
  xq cols 0:2048   -> h2   (feeds cnt_o, K=2048, out v = 2s+1)

Sharding: cores 0-3 compute even output rows u=2t (host supplies the +
row-fold), cores 4-7 odd rows u=2t+1 (the - row-fold). All matmuls bf16
with fp32 PSUM. The host applies a pure index permutation (vmap) when
assembling the final array.
"""

import sys

for _p in ("/opt/trn_rl_repo", "/opt/pypackages"):
    if _p not in sys.path:
        sys.path.append(_p)

import numpy as np

M = 4096
N = 4096
H = M // 2          # 2048: pass-1 contraction length
N_CORES = 8
TS = 512            # t-shard width per core (512 output rows per core)

_CACHE = {}


def _build_nc():
    import concourse.bacc as bacc
    import concourse.mybir as mybir
    from concourse import tile

    BF16 = mybir.dt.bfloat16
    F32 = mybir.dt.float32

    nc = bacc.Bacc("TRN2", target_bir_lowering=False, debug=False,
                   num_devices=N_CORES)
    xr = nc.dram_tensor("xr", [H, 2048], BF16, kind="ExternalInput")
    xq = nc.dram_tensor("xq", [H, 2048], BF16, kind="ExternalInput")
    cmt = nc.dram_tensor("cmt", [H, TS], BF16, kind="ExternalInput")
    cne = nc.dram_tensor("cne", [512, 1024], BF16, kind="ExternalInput")
    ceo = nc.dram_tensor("ceo", [1024, 1024], BF16, kind="ExternalInput")
    cnto = nc.dram_tensor("cnto", [H, H], BF16, kind="ExternalInput")
    out = nc.dram_tensor("out", [TS, N], BF16, kind="ExternalOutput")

    with tile.TileContext(nc) as tc:
        with (
            tc.tile_pool(name="persist", bufs=1) as persist,
            tc.tile_pool(name="stream", bufs=6) as stream,
            tc.tile_pool(name="ctpool", bufs=8) as ctpool,
            tc.tile_pool(name="otpool", bufs=4) as otpool,
            tc.tile_pool(name="psum", bufs=8, space="PSUM") as pp,
        ):
            cmt_sb = [persist.tile([128, TS], BF16, tag=f"cmt{j}",
                                   name=f"cmt_sb{j}")
                      for j in range(16)]

            # pass-1 outputs: 0-3 gg3p, 4-7 gg3m, 8-15 hh2, 16-31 h2
            a_sb = [persist.tile([128, TS], BF16, tag=f"a{cc}",
                                 name=f"a_sb{cc}")
                    for cc in range(32)]
            cne_sb = [persist.tile([128, 1024], BF16, tag=f"cne{j}",
                                   name=f"cne_sb{j}")
                      for j in range(4)]

            # ---- pass 1: A[c, t] = sum_{i<H} src[i, c] * cmt[i, t]
            # 512-wide c-groups use 4 PSUM banks each, so a group's PSUM
            # evacuation overlaps the next group's matmuls
            srcs = [(xr, 0), (xr, 512), (xr, 1024), (xr, 1536),
                    (xq, 0), (xq, 512), (xq, 1024), (xq, 1536)]
            for sg in range(8):
                src, coff = srcs[sg]
                ps = [pp.tile([128, TS], F32, tag="ps", name=f"ps1_{sg}_{i}")
                      for i in range(4)]
                for j in range(16):      # contraction chunks over i
                    gj = stream.tile([128, 512], BF16, tag="gj")
                    if sg == 0 and j == 0:
                        # fine-grained first sliver: small loads unblock the
                        # first matmul ASAP
                        nc.sync.dma_start(cmt_sb[0][:], cmt[0:128, :])
                        nc.sync.dma_start(gj[:, 0:128], xr[0:128, 0:128])
                        nc.sync.dma_start(gj[:, 128:512], xr[0:128, 128:512])
                    else:
                        nc.sync.dma_start(
                            gj[:], src[j * 128:(j + 1) * 128,
                                       coff:coff + 512])
                        if sg == 0:
                            # lazy constant loads: first tiles aren't stuck
                            # behind a bulk preload at kernel start
                            nc.sync.dma_start(cmt_sb[j][:],
                                              cmt[j * 128:(j + 1) * 128, :])
                        if sg == 4 and j < 4:
                            nc.sync.dma_start(cne_sb[j][:],
                                              cne[j * 128:(j + 1) * 128, :])
                    for cs in range(4):
                        nc.tensor.matmul(
                            ps[cs][:],
                            gj[:, cs * 128:(cs + 1) * 128],
                            cmt_sb[j][:],
                            start=(j == 0), stop=(j == 15))
                for cs in range(4):
                    if cs % 2 == 0:
                        nc.vector.tensor_copy(a_sb[sg * 4 + cs][:], ps[cs][:])
                    else:
                        nc.scalar.copy(a_sb[sg * 4 + cs][:], ps[cs][:])

            # ---- pass 2 branch A/B (ee_e, ee_o): K=512
            #   out[t, 0:512]    = sum_c gg3p[c,t] ee_e[c,r]   (v = 8r)
            #   out[t, 512:1024] = sum_c gg3m[c,t] ee_o[c,r]   (v = 8r+4)
            pe1 = [pp.tile([128, 512], F32, tag="ps", name=f"pe1_{i}")
                   for i in range(4)]
            pe2 = [pp.tile([128, 512], F32, tag="ps", name=f"pe2_{i}")
                   for i in range(4)]
            for ch in range(4):
                for us in range(4):
                    nc.tensor.matmul(
                        pe1[us][:],
                        a_sb[ch][:, us * 128:(us + 1) * 128],
                        cne_sb[ch][:, 0:512],
                        start=(ch == 0), stop=(ch == 3))
                    nc.tensor.matmul(
                        pe2[us][:],
                        a_sb[4 + ch][:, us * 128:(us + 1) * 128],
                        cne_sb[ch][:, 512:1024],
                        start=(ch == 0), stop=(ch == 3))
            for us in range(4):
                ot = otpool.tile([128, 1024], BF16, tag="ot")
                nc.vector.tensor_copy(ot[:, 0:512], pe1[us][:])
                nc.scalar.copy(ot[:, 512:1024], pe2[us][:])
                nc.sync.dma_start(
                    out[us * 128:(us + 1) * 128, 0:1024], ot[:])

            # ---- pass 2 branch C (eo): K=1024, out[t, 1024:2048] (v = 4r+2)
            po = [pp.tile([128, 512], F32, tag="ps", name=f"po_{i}")
                  for i in range(8)]
            for ch in range(8):
                ct = ctpool.tile([128, 1024], BF16, tag="ct")
                nc.sync.dma_start(ct[:], ceo[ch * 128:(ch + 1) * 128, :])
                for half in range(2):
                    for us in range(4):
                        nc.tensor.matmul(
                            po[half * 4 + us][:],
                            a_sb[8 + ch][:, us * 128:(us + 1) * 128],
                            ct[:, half * 512:(half + 1) * 512],
                            start=(ch == 0), stop=(ch == 7))
            for us in range(4):
                ot = otpool.tile([128, 1024], BF16, tag="ot")
                nc.vector.tensor_copy(ot[:, 0:512], po[us][:])
                nc.scalar.copy(ot[:, 512:1024], po[4 + us][:])
                nc.sync.dma_start(
                    out[us * 128:(us + 1) * 128, 1024:2048], ot[:])

            # ---- pass 2 branch D (odd v): K=2048, out[t, 2s+1]
            for sgp in range(2):         # pairs of 512-wide s-groups
                pd = [pp.tile([128, 512], F32, tag="ps",
                              name=f"pd_{sgp}_{i}") for i in range(8)]
                for cc in range(16):     # contraction chunks over c < H
                    cto = ctpool.tile([128, 1024], BF16, tag="ct")
                    nc.sync.dma_start(
                        cto[:], cnto[cc * 128:(cc + 1) * 128,
                                     sgp * 1024:(sgp + 1) * 1024])
                    for half in range(2):
                        for us in range(4):
                            nc.tensor.matmul(
                                pd[half * 4 + us][:],
                                a_sb[16 + cc][:, us * 128:(us + 1) * 128],
                                cto[:, half * 512:(half + 1) * 512],
                                start=(cc == 0), stop=(cc == 15))
                for us in range(4):
                    ot = otpool.tile([128, 1024], BF16, tag="ot")
                    nc.vector.tensor_copy(ot[:, 0:512], pd[us][:])
                    nc.scalar.copy(ot[:, 512:1024], pd[4 + us][:])
                    nc.scalar.dma_start(
                        out[us * 128:(us + 1) * 128,
                            H + sgp * 1024:H + (sgp + 1) * 1024], ot[:])
    nc.finalize()
    return nc


def _consts():
    """Host-precomputed constant operands (input-independent)."""
    import ml_dtypes
    bf16 = ml_dtypes.bfloat16
    i = np.arange(H, dtype=np.float64)[:, None]
    t = np.arange(H, dtype=np.float64)[None, :]
    ce = np.cos(np.pi * (2 * i + 1) * (2 * t) / (2 * M))       # [i<H, t<H]
    co = np.cos(np.pi * (2 * i + 1) * (2 * t + 1) / (2 * M))
    cmt_e = ce.astype(bf16)                                     # [H, H]
    cmt_o = co.astype(bf16)

    # pass-2 constants
    q5 = np.arange(512, dtype=np.float64)[:, None]
    r5 = np.arange(512, dtype=np.float64)[None, :]
    ee_e = np.cos(np.pi * (2 * q5 + 1) * r5 / 1024)             # [512, 512]
    ee_o = np.cos(np.pi * (2 * q5 + 1) * (2 * r5 + 1) / 2048)
    cne = np.ascontiguousarray(
        np.concatenate([ee_e, ee_o], axis=1).astype(bf16))      # [512, 1024]

    qa = np.arange(1024, dtype=np.float64)[:, None]
    ra = np.arange(1024, dtype=np.float64)[None, :]
    eo = np.cos(np.pi * (2 * qa + 1) * (2 * ra + 1) / 4096)
    ceo = np.ascontiguousarray(eo.astype(bf16))                 # [1024, 1024]

    cnto = np.ascontiguousarray(co[:, :H].astype(bf16))         # [H, H]

    # device output column j -> final v permutation
    vmap = np.empty(N, dtype=np.int64)
    j5 = np.arange(512)
    ja = np.arange(1024)
    jh = np.arange(2048)
    vmap[0:512] = 8 * j5
    vmap[512:1024] = 8 * j5 + 4
    vmap[1024:2048] = 4 * ja + 2
    vmap[2048:4096] = 2 * jh + 1
    return cmt_e, cmt_o, cne, ceo, cnto, vmap


def _fold_cols(a):
    """One symmetric/antisymmetric column-fold level: returns (+, -)."""
    w = a.shape[1] // 2
    lo = a[:, :w]
    hi = a[:, w:][:, ::-1]
    return lo + hi, lo - hi


def _run_res(x_np, trace=False):
    from concourse.bass_utils import run_bass_kernel_spmd
    import ml_dtypes
    bf16 = ml_dtypes.bfloat16

    if "nc" not in _CACHE:
        _CACHE["nc"] = _build_nc()
        _CACHE["consts"] = _consts()
    nc = _CACHE["nc"]
    cmt_e, cmt_o, cne, ceo, cnto, vmap = _CACHE["consts"]

    x_np = np.asarray(x_np, dtype=np.float32)
    xtop = x_np[:H]
    xbot = x_np[M - 1:H - 1:-1]

    ins = []
    for par in range(2):
        gp = xtop + xbot if par == 0 else xtop - xbot      # [2048, 4096]
        gE, gO = _fold_cols(gp)                            # [2048, 2048]
        gEE, gEO = _fold_cols(gE)                          # [2048, 1024]
        g3P, g3M = _fold_cols(gEE)                         # [2048, 512]
        xr = np.ascontiguousarray(
            np.concatenate([g3P, g3M, gEO], axis=1).astype(bf16))
        xq = np.ascontiguousarray(gO.astype(bf16))
        ins.append((xr, xq))

    in_maps = []
    for k in range(N_CORES):
        par = 0 if k < 4 else 1
        ksh = k % 4
        cm = cmt_e if par == 0 else cmt_o
        xr, xq = ins[par]
        in_maps.append({
            "xr": xr,
            "xq": xq,
            "cmt": np.ascontiguousarray(cm[:, ksh * TS:(ksh + 1) * TS]),
            "cne": cne,
            "ceo": ceo,
            "cnto": cnto,
        })
    res = run_bass_kernel_spmd(nc, in_maps, core_ids=list(range(N_CORES)),
                               trace=trace)

    out = np.empty((M, N), dtype=np.float32)
    tmp = np.empty((TS, N), dtype=np.float32)
    for k in range(N_CORES):
        r = np.asarray(res.results[k]["out"], dtype=np.float32)
        par = 0 if k < 4 else 1
        t0 = (k % 4) * TS
        rows = slice(2 * t0 + par, 2 * (t0 + TS) + par, 2)
        tmp[:, vmap] = r
        out[rows] = tmp
    return out, res.exec_time_ns, res


def kernel(x):
    out, _, _ = _run_res(np.asarray(x), trace=False)
    return out


# revision 8
# speedup vs baseline: 1.0420x; 1.0420x over previous
"""2D DCT-II (4096x4096, f32) on 8 Trainium2 NeuronCores.

out = Cm @ x @ Cn^T with Cm[u,i] = cos(pi*(2i+1)*u/(2M)) — mathematically
identical to the reference's Makhoul-FFT formulation.

Pass 1 contracts over x's row index i (K=2048 after the host row-fold).
Pass 2 contracts over x's column index c. Every c-axis fold that pass 2
needs (g2/h2 level-1, gg2/hh2 level-2, and a level-3 fold of the ee
branch) distributes over pass 1's matmul and is therefore applied by the
HOST to the pass-1 input columns: A^T[c,t] +/- A^T[C-1-c,t] =
sum_i (g[i,c] +/- g[i,C-1-c]) cmt[i,t]. The device never reverses or
folds anything — pass 1 directly emits the folded operands pass 2 needs:
  xr cols 0:512    -> gg3p (feeds ee_e, K=512, out v = 8r)
  xr cols 512:1024 -> gg3m (feeds ee_o, K=512, out v = 8r+4)
  xr cols 1024:2048-> hh2  (feeds eo,   K=1024, out v = 4r+2)
  xq cols 0:2048   -> h2   (feeds cnt_o, K=2048, out v = 2s+1)

Sharding: cores 0-3 compute even output rows u=2t (host supplies the +
row-fold), cores 4-7 odd rows u=2t+1 (the - row-fold). All matmuls bf16
with fp32 PSUM. The host applies a pure index permutation (vmap) when
assembling the final array.
"""

import sys

for _p in ("/opt/trn_rl_repo", "/opt/pypackages"):
    if _p not in sys.path:
        sys.path.append(_p)

import numpy as np

M = 4096
N = 4096
H = M // 2          # 2048: pass-1 contraction length
N_CORES = 8
TS = 512            # t-shard width per core (512 output rows per core)

_CACHE = {}


def _build_nc():
    import concourse.bacc as bacc
    import concourse.mybir as mybir
    from concourse import tile

    BF16 = mybir.dt.bfloat16
    F32 = mybir.dt.float32

    nc = bacc.Bacc("TRN2", target_bir_lowering=False, debug=False,
                   num_devices=N_CORES)
    xr = nc.dram_tensor("xr", [H, 2048], BF16, kind="ExternalInput")
    xq = nc.dram_tensor("xq", [H, 2048], BF16, kind="ExternalInput")
    cmt = nc.dram_tensor("cmt", [H, TS], BF16, kind="ExternalInput")
    cne = nc.dram_tensor("cne", [512, 1024], BF16, kind="ExternalInput")
    ceo = nc.dram_tensor("ceo", [1024, 1024], BF16, kind="ExternalInput")
    cnto = nc.dram_tensor("cnto", [H, H], BF16, kind="ExternalInput")
    out = nc.dram_tensor("out", [TS, N], BF16, kind="ExternalOutput")

    with tile.TileContext(nc) as tc:
        with (
            tc.tile_pool(name="persist", bufs=1) as persist,
            tc.tile_pool(name="stream", bufs=6) as stream,
            tc.tile_pool(name="ctpool", bufs=8) as ctpool,
            tc.tile_pool(name="otpool", bufs=4) as otpool,
            tc.tile_pool(name="psum", bufs=8, space="PSUM") as pp,
        ):
            cmt_sb = [persist.tile([128, TS], BF16, tag=f"cmt{j}",
                                   name=f"cmt_sb{j}")
                      for j in range(16)]

            # pass-1 outputs: 0-3 gg3p, 4-7 gg3m, 8-15 hh2, 16-31 h2
            a_sb = [persist.tile([128, TS], BF16, tag=f"a{cc}",
                                 name=f"a_sb{cc}")
                    for cc in range(32)]
            cne_sb = [persist.tile([128, 1024], BF16, tag=f"cne{j}",
                                   name=f"cne_sb{j}")
                      for j in range(4)]

            # ---- pass 1: A[c, t] = sum_{i<H} src[i, c] * cmt[i, t]
            srcs = [(xr, 0), (xr, 1024), (xq, 0), (xq, 1024)]
            for cg in range(4):
                src, coff = srcs[cg]
                ps = [pp.tile([128, TS], F32, tag="ps", name=f"ps1_{cg}_{i}")
                      for i in range(8)]
                for j in range(16):      # contraction chunks over i
                    gj = stream.tile([128, 1024], BF16, tag="gj")
                    if cg == 0 and j == 0:
                        # fine-grained first sliver: small loads unblock the
                        # first matmul ASAP; constants ride the idle gpsimd
                        # DMA queue so sync stays dedicated to gj streaming
                        nc.gpsimd.dma_start(cmt_sb[0][:], cmt[0:128, :])
                        nc.sync.dma_start(gj[:, 0:128], xr[0:128, 0:128])
                        nc.sync.dma_start(gj[:, 128:1024], xr[0:128, 128:1024])
                    else:
                        nc.sync.dma_start(
                            gj[:], src[j * 128:(j + 1) * 128,
                                       coff:coff + 1024])
                        if cg == 0:
                            # lazy constant loads: first tiles aren't stuck
                            # behind a bulk preload at kernel start
                            nc.gpsimd.dma_start(cmt_sb[j][:],
                                                cmt[j * 128:(j + 1) * 128, :])
                        if cg == 1 and j < 4:
                            nc.gpsimd.dma_start(cne_sb[j][:],
                                                cne[j * 128:(j + 1) * 128, :])
                    for cs in range(8):
                        nc.tensor.matmul(
                            ps[cs][:],
                            gj[:, cs * 128:(cs + 1) * 128],
                            cmt_sb[j][:],
                            start=(j == 0), stop=(j == 15))
                for cs in range(8):
                    if cs % 2 == 0:
                        nc.vector.tensor_copy(a_sb[cg * 8 + cs][:], ps[cs][:])
                    else:
                        nc.scalar.copy(a_sb[cg * 8 + cs][:], ps[cs][:])

            # ---- pass 2 branch A/B (ee_e, ee_o): K=512
            #   out[t, 0:512]    = sum_c gg3p[c,t] ee_e[c,r]   (v = 8r)
            #   out[t, 512:1024] = sum_c gg3m[c,t] ee_o[c,r]   (v = 8r+4)
            pe1 = [pp.tile([128, 512], F32, tag="ps", name=f"pe1_{i}")
                   for i in range(4)]
            pe2 = [pp.tile([128, 512], F32, tag="ps", name=f"pe2_{i}")
                   for i in range(4)]
            for ch in range(4):
                for us in range(4):
                    nc.tensor.matmul(
                        pe1[us][:],
                        a_sb[ch][:, us * 128:(us + 1) * 128],
                        cne_sb[ch][:, 0:512],
                        start=(ch == 0), stop=(ch == 3))
                    nc.tensor.matmul(
                        pe2[us][:],
                        a_sb[4 + ch][:, us * 128:(us + 1) * 128],
                        cne_sb[ch][:, 512:1024],
                        start=(ch == 0), stop=(ch == 3))
            for us in range(4):
                ot = otpool.tile([128, 1024], BF16, tag="ot")
                nc.vector.tensor_copy(ot[:, 0:512], pe1[us][:])
                nc.scalar.copy(ot[:, 512:1024], pe2[us][:])
                nc.sync.dma_start(
                    out[us * 128:(us + 1) * 128, 0:1024], ot[:])

            # ---- pass 2 branch C (eo): K=1024, out[t, 1024:2048] (v = 4r+2)
            po = [pp.tile([128, 512], F32, tag="ps", name=f"po_{i}")
                  for i in range(8)]
            for ch in range(8):
                ct = ctpool.tile([128, 1024], BF16, tag="ct")
                nc.sync.dma_start(ct[:], ceo[ch * 128:(ch + 1) * 128, :])
                for half in range(2):
                    for us in range(4):
                        nc.tensor.matmul(
                            po[half * 4 + us][:],
                            a_sb[8 + ch][:, us * 128:(us + 1) * 128],
                            ct[:, half * 512:(half + 1) * 512],
                            start=(ch == 0), stop=(ch == 7))
            for us in range(4):
                ot = otpool.tile([128, 1024], BF16, tag="ot")
                nc.vector.tensor_copy(ot[:, 0:512], po[us][:])
                nc.scalar.copy(ot[:, 512:1024], po[4 + us][:])
                nc.sync.dma_start(
                    out[us * 128:(us + 1) * 128, 1024:2048], ot[:])

            # ---- pass 2 branch D (odd v): K=2048, out[t, 2s+1]
            for sgp in range(2):         # pairs of 512-wide s-groups
                pd = [pp.tile([128, 512], F32, tag="ps",
                              name=f"pd_{sgp}_{i}") for i in range(8)]
                for cc in range(16):     # contraction chunks over c < H
                    cto = ctpool.tile([128, 1024], BF16, tag="ct")
                    nc.sync.dma_start(
                        cto[:], cnto[cc * 128:(cc + 1) * 128,
                                     sgp * 1024:(sgp + 1) * 1024])
                    for half in range(2):
                        for us in range(4):
                            nc.tensor.matmul(
                                pd[half * 4 + us][:],
                                a_sb[16 + cc][:, us * 128:(us + 1) * 128],
                                cto[:, half * 512:(half + 1) * 512],
                                start=(cc == 0), stop=(cc == 15))
                for us in range(4):
                    ot = otpool.tile([128, 1024], BF16, tag="ot")
                    nc.vector.tensor_copy(ot[:, 0:512], pd[us][:])
                    nc.scalar.copy(ot[:, 512:1024], pd[4 + us][:])
                    nc.scalar.dma_start(
                        out[us * 128:(us + 1) * 128,
                            H + sgp * 1024:H + (sgp + 1) * 1024], ot[:])
    nc.finalize()
    return nc


def _consts():
    """Host-precomputed constant operands (input-independent)."""
    import ml_dtypes
    bf16 = ml_dtypes.bfloat16
    i = np.arange(H, dtype=np.float64)[:, None]
    t = np.arange(H, dtype=np.float64)[None, :]
    ce = np.cos(np.pi * (2 * i + 1) * (2 * t) / (2 * M))       # [i<H, t<H]
    co = np.cos(np.pi * (2 * i + 1) * (2 * t + 1) / (2 * M))
    cmt_e = ce.astype(bf16)                                     # [H, H]
    cmt_o = co.astype(bf16)

    # pass-2 constants
    q5 = np.arange(512, dtype=np.float64)[:, None]
    r5 = np.arange(512, dtype=np.float64)[None, :]
    ee_e = np.cos(np.pi * (2 * q5 + 1) * r5 / 1024)             # [512, 512]
    ee_o = np.cos(np.pi * (2 * q5 + 1) * (2 * r5 + 1) / 2048)
    cne = np.ascontiguousarray(
        np.concatenate([ee_e, ee_o], axis=1).astype(bf16))      # [512, 1024]

    qa = np.arange(1024, dtype=np.float64)[:, None]
    ra = np.arange(1024, dtype=np.float64)[None, :]
    eo = np.cos(np.pi * (2 * qa + 1) * (2 * ra + 1) / 4096)
    ceo = np.ascontiguousarray(eo.astype(bf16))                 # [1024, 1024]

    cnto = np.ascontiguousarray(co[:, :H].astype(bf16))         # [H, H]

    # device output column j -> final v permutation
    vmap = np.empty(N, dtype=np.int64)
    j5 = np.arange(512)
    ja = np.arange(1024)
    jh = np.arange(2048)
    vmap[0:512] = 8 * j5
    vmap[512:1024] = 8 * j5 + 4
    vmap[1024:2048] = 4 * ja + 2
    vmap[2048:4096] = 2 * jh + 1
    return cmt_e, cmt_o, cne, ceo, cnto, vmap


def _fold_cols(a):
    """One symmetric/antisymmetric column-fold level: returns (+, -)."""
    w = a.shape[1] // 2
    lo = a[:, :w]
    hi = a[:, w:][:, ::-1]
    return lo + hi, lo - hi


def _run_res(x_np, trace=False):
    from concourse.bass_utils import run_bass_kernel_spmd
    import ml_dtypes
    bf16 = ml_dtypes.bfloat16

    if "nc" not in _CACHE:
        _CACHE["nc"] = _build_nc()
        _CACHE["consts"] = _consts()
    nc = _CACHE["nc"]
    cmt_e, cmt_o, cne, ceo, cnto, vmap = _CACHE["consts"]

    x_np = np.asarray(x_np, dtype=np.float32)
    xtop = x_np[:H]
    xbot = x_np[M - 1:H - 1:-1]

    ins = []
    for par in range(2):
        gp = xtop + xbot if par == 0 else xtop - xbot      # [2048, 4096]
        gE, gO = _fold_cols(gp)                            # [2048, 2048]
        gEE, gEO = _fold_cols(gE)                          # [2048, 1024]
        g3P, g3M = _fold_cols(gEE)                         # [2048, 512]
        xr = np.ascontiguousarray(
            np.concatenate([g3P, g3M, gEO], axis=1).astype(bf16))
        xq = np.ascontiguousarray(gO.astype(bf16))
        ins.append((xr, xq))

    in_maps = []
    for k in range(N_CORES):
        par = 0 if k < 4 else 1
        ksh = k % 4
        cm = cmt_e if par == 0 else cmt_o
        xr, xq = ins[par]
        in_maps.append({
            "xr": xr,
            "xq": xq,
            "cmt": np.ascontiguousarray(cm[:, ksh * TS:(ksh + 1) * TS]),
            "cne": cne,
            "ceo": ceo,
            "cnto": cnto,
        })
    res = run_bass_kernel_spmd(nc, in_maps, core_ids=list(range(N_CORES)),
                               trace=trace)

    out = np.empty((M, N), dtype=np.float32)
    tmp = np.empty((TS, N), dtype=np.float32)
    for k in range(N_CORES):
        r = np.asarray(res.results[k]["out"], dtype=np.float32)
        par = 0 if k < 4 else 1
        t0 = (k % 4) * TS
        rows = slice(2 * t0 + par, 2 * (t0 + TS) + par, 2)
        tmp[:, vmap] = r
        out[rows] = tmp
    return out, res.exec_time_ns, res


def kernel(x):
    out, _, _ = _run_res(np.asarray(x), trace=False)
    return out


# revision 9
# speedup vs baseline: 1.0573x; 1.0147x over previous
"""2D DCT-II (4096x4096, f32) on 8 Trainium2 NeuronCores.

out = Cm @ x @ Cn^T with Cm[u,i] = cos(pi*(2i+1)*u/(2M)) — mathematically
identical to the reference's Makhoul-FFT formulation.

Pass 1 contracts over x's row index i (K=2048 after the host row-fold).
Pass 2 contracts over x's column index c. Every c-axis fold that pass 2
needs (g2/h2 level-1, gg2/hh2 level-2, and a level-3 fold of the ee
branch) distributes over pass 1's matmul and is therefore applied by the
HOST to the pass-1 input columns: A^T[c,t] +/- A^T[C-1-c,t] =
sum_i (g[i,c] +/- g[i,C-1-c]) cmt[i,t]. The device never reverses or
folds anything — pass 1 directly emits the folded operands pass 2 needs:
  xr cols 0:512    -> gg3p (feeds ee_e, K=512, out v = 8r)
  xr cols 512:1024 -> gg3m (feeds ee_o, K=512, out v = 8r+4)
  xr cols 1024:2048-> hh2  (feeds eo,   K=1024, out v = 4r+2)
  xq cols 0:2048   -> h2   (feeds cnt_o, K=2048, out v = 2s+1)

Sharding: cores 0-3 compute even output rows u=2t (host supplies the +
row-fold), cores 4-7 odd rows u=2t+1 (the - row-fold). All matmuls bf16
with fp32 PSUM. The host applies a pure index permutation (vmap) when
assembling the final array.
"""

import sys

for _p in ("/opt/trn_rl_repo", "/opt/pypackages"):
    if _p not in sys.path:
        sys.path.append(_p)

import numpy as np

M = 4096
N = 4096
H = M // 2          # 2048: pass-1 contraction length
N_CORES = 8
TS = 512            # t-shard width per core (512 output rows per core)

_CACHE = {}


def _build_nc():
    import concourse.bacc as bacc
    import concourse.mybir as mybir
    from concourse import tile

    BF16 = mybir.dt.bfloat16
    F32 = mybir.dt.float32

    nc = bacc.Bacc("TRN2", target_bir_lowering=False, debug=False,
                   num_devices=N_CORES)
    xr = nc.dram_tensor("xr", [H, 2048], BF16, kind="ExternalInput")
    xq = nc.dram_tensor("xq", [H, 2048], BF16, kind="ExternalInput")
    cmt = nc.dram_tensor("cmt", [H, TS], BF16, kind="ExternalInput")
    cne = nc.dram_tensor("cne", [512, 1024], BF16, kind="ExternalInput")
    ceo = nc.dram_tensor("ceo", [1024, 1024], BF16, kind="ExternalInput")
    cnto = nc.dram_tensor("cnto", [H, H], BF16, kind="ExternalInput")
    out = nc.dram_tensor("out", [TS, N], BF16, kind="ExternalOutput")

    with tile.TileContext(nc) as tc:
        with (
            tc.tile_pool(name="persist", bufs=1) as persist,
            tc.tile_pool(name="stream", bufs=6) as stream,
            tc.tile_pool(name="ctpool", bufs=8) as ctpool,
            tc.tile_pool(name="otpool", bufs=4) as otpool,
            tc.tile_pool(name="psum", bufs=8, space="PSUM") as pp,
        ):
            cmt_sb = [persist.tile([128, TS], BF16, tag=f"cmt{j}",
                                   name=f"cmt_sb{j}")
                      for j in range(16)]

            # pass-1 outputs: 0-3 gg3p, 4-7 gg3m, 8-15 hh2, 16-31 h2
            a_sb = [persist.tile([128, TS], BF16, tag=f"a{cc}",
                                 name=f"a_sb{cc}")
                    for cc in range(32)]
            cne_sb = [persist.tile([128, 1024], BF16, tag=f"cne{j}",
                                   name=f"cne_sb{j}")
                      for j in range(4)]

            # ---- pass 1: A[c, t] = sum_{i<H} src[i, c] * cmt[i, t]
            srcs = [(xr, 0), (xr, 1024), (xq, 0), (xq, 1024)]
            for cg in range(4):
                src, coff = srcs[cg]
                ps = [pp.tile([128, TS], F32, tag="ps", name=f"ps1_{cg}_{i}")
                      for i in range(8)]
                for j in range(16):      # contraction chunks over i
                    gj = stream.tile([128, 1024], BF16, tag="gj")
                    if cg == 0 and j == 0:
                        # fine-grained first sliver: small loads unblock the
                        # first matmul ASAP
                        nc.sync.dma_start(cmt_sb[0][:], cmt[0:128, :])
                        nc.sync.dma_start(gj[:, 0:128], xr[0:128, 0:128])
                        nc.sync.dma_start(gj[:, 128:1024], xr[0:128, 128:1024])
                    else:
                        nc.sync.dma_start(
                            gj[:], src[j * 128:(j + 1) * 128,
                                       coff:coff + 1024])
                        if cg == 0:
                            # lazy constant loads: first tiles aren't stuck
                            # behind a bulk preload at kernel start
                            nc.sync.dma_start(cmt_sb[j][:],
                                              cmt[j * 128:(j + 1) * 128, :])
                        if cg == 1 and j < 4:
                            nc.sync.dma_start(cne_sb[j][:],
                                              cne[j * 128:(j + 1) * 128, :])
                    for cs in range(8):
                        nc.tensor.matmul(
                            ps[cs][:],
                            gj[:, cs * 128:(cs + 1) * 128],
                            cmt_sb[j][:],
                            start=(j == 0), stop=(j == 15))
                for cs in range(8):
                    if cs % 2 == 0:
                        nc.vector.tensor_copy(a_sb[cg * 8 + cs][:], ps[cs][:])
                    else:
                        nc.scalar.copy(a_sb[cg * 8 + cs][:], ps[cs][:])

            # ---- pass 2 branch A/B (ee_e, ee_o): K=512
            #   out[t, 0:512]    = sum_c gg3p[c,t] ee_e[c,r]   (v = 8r)
            #   out[t, 512:1024] = sum_c gg3m[c,t] ee_o[c,r]   (v = 8r+4)
            pe1 = [pp.tile([128, 512], F32, tag="ps", name=f"pe1_{i}")
                   for i in range(4)]
            pe2 = [pp.tile([128, 512], F32, tag="ps", name=f"pe2_{i}")
                   for i in range(4)]
            for ch in range(4):
                for us in range(4):
                    nc.tensor.matmul(
                        pe1[us][:],
                        a_sb[ch][:, us * 128:(us + 1) * 128],
                        cne_sb[ch][:, 0:512],
                        start=(ch == 0), stop=(ch == 3))
                    nc.tensor.matmul(
                        pe2[us][:],
                        a_sb[4 + ch][:, us * 128:(us + 1) * 128],
                        cne_sb[ch][:, 512:1024],
                        start=(ch == 0), stop=(ch == 3))
            for us in range(4):
                ot = otpool.tile([128, 1024], BF16, tag="ot")
                nc.vector.tensor_copy(ot[:, 0:512], pe1[us][:])
                nc.scalar.copy(ot[:, 512:1024], pe2[us][:])
                nc.sync.dma_start(
                    out[us * 128:(us + 1) * 128, 0:1024], ot[:])

            # ---- pass 2 branch C (eo): K=1024, out[t, 1024:2048] (v = 4r+2)
            po = [pp.tile([128, 512], F32, tag="ps", name=f"po_{i}")
                  for i in range(8)]
            for ch in range(8):
                ct = ctpool.tile([128, 1024], BF16, tag="ct")
                nc.sync.dma_start(ct[:], ceo[ch * 128:(ch + 1) * 128, :])
                for half in range(2):
                    for us in range(4):
                        nc.tensor.matmul(
                            po[half * 4 + us][:],
                            a_sb[8 + ch][:, us * 128:(us + 1) * 128],
                            ct[:, half * 512:(half + 1) * 512],
                            start=(ch == 0), stop=(ch == 7))
            for us in range(4):
                ot = otpool.tile([128, 1024], BF16, tag="ot")
                nc.vector.tensor_copy(ot[:, 0:512], po[us][:])
                nc.scalar.copy(ot[:, 512:1024], po[4 + us][:])
                nc.sync.dma_start(
                    out[us * 128:(us + 1) * 128, 1024:2048], ot[:])

            # ---- pass 2 branch D (odd v): K=2048, out[t, 2s+1]
            for sgp in range(2):         # pairs of 512-wide s-groups
                pd = [pp.tile([128, 512], F32, tag="ps",
                              name=f"pd_{sgp}_{i}") for i in range(8)]
                for cc in range(16):     # contraction chunks over c < H
                    cto = ctpool.tile([128, 1024], BF16, tag="ct")
                    nc.sync.dma_start(
                        cto[:], cnto[cc * 128:(cc + 1) * 128,
                                     sgp * 1024:(sgp + 1) * 1024])
                    for half in range(2):
                        for us in range(4):
                            nc.tensor.matmul(
                                pd[half * 4 + us][:],
                                a_sb[16 + cc][:, us * 128:(us + 1) * 128],
                                cto[:, half * 512:(half + 1) * 512],
                                start=(cc == 0), stop=(cc == 15))
                for us in range(4):
                    ot = otpool.tile([128, 1024], BF16, tag="ot")
                    nc.vector.tensor_copy(ot[:, 0:512], pd[us][:])
                    nc.scalar.copy(ot[:, 512:1024], pd[4 + us][:])
                    nc.scalar.dma_start(
                        out[us * 128:(us + 1) * 128,
                            H + sgp * 1024:H + (sgp + 1) * 1024], ot[:])
    nc.finalize()
    return nc


def _consts():
    """Host-precomputed constant operands (input-independent)."""
    import ml_dtypes
    bf16 = ml_dtypes.bfloat16
    i = np.arange(H, dtype=np.float64)[:, None]
    t = np.arange(H, dtype=np.float64)[None, :]
    ce = np.cos(np.pi * (2 * i + 1) * (2 * t) / (2 * M))       # [i<H, t<H]
    co = np.cos(np.pi * (2 * i + 1) * (2 * t + 1) / (2 * M))
    cmt_e = ce.astype(bf16)                                     # [H, H]
    cmt_o = co.astype(bf16)

    # pass-2 constants
    q5 = np.arange(512, dtype=np.float64)[:, None]
    r5 = np.arange(512, dtype=np.float64)[None, :]
    ee_e = np.cos(np.pi * (2 * q5 + 1) * r5 / 1024)             # [512, 512]
    ee_o = np.cos(np.pi * (2 * q5 + 1) * (2 * r5 + 1) / 2048)
    cne = np.ascontiguousarray(
        np.concatenate([ee_e, ee_o], axis=1).astype(bf16))      # [512, 1024]

    qa = np.arange(1024, dtype=np.float64)[:, None]
    ra = np.arange(1024, dtype=np.float64)[None, :]
    eo = np.cos(np.pi * (2 * qa + 1) * (2 * ra + 1) / 4096)
    ceo = np.ascontiguousarray(eo.astype(bf16))                 # [1024, 1024]

    cnto = np.ascontiguousarray(co[:, :H].astype(bf16))         # [H, H]

    # device output column j -> final v permutation
    vmap = np.empty(N, dtype=np.int64)
    j5 = np.arange(512)
    ja = np.arange(1024)
    jh = np.arange(2048)
    vmap[0:512] = 8 * j5
    vmap[512:1024] = 8 * j5 + 4
    vmap[1024:2048] = 4 * ja + 2
    vmap[2048:4096] = 2 * jh + 1
    return cmt_e, cmt_o, cne, ceo, cnto, vmap


def _fold_cols(a):
    """One symmetric/antisymmetric column-fold level: returns (+, -)."""
    w = a.shape[1] // 2
    lo = a[:, :w]
    hi = a[:, w:][:, ::-1]
    return lo + hi, lo - hi


def _run_res(x_np, trace=False):
    from concourse.bass_utils import run_bass_kernel_spmd
    import ml_dtypes
    bf16 = ml_dtypes.bfloat16

    if "nc" not in _CACHE:
        _CACHE["nc"] = _build_nc()
        _CACHE["consts"] = _consts()
    nc = _CACHE["nc"]
    cmt_e, cmt_o, cne, ceo, cnto, vmap = _CACHE["consts"]

    x_np = np.asarray(x_np, dtype=np.float32)
    xtop = x_np[:H]
    xbot = x_np[M - 1:H - 1:-1]

    ins = []
    for par in range(2):
        gp = xtop + xbot if par == 0 else xtop - xbot      # [2048, 4096]
        gE, gO = _fold_cols(gp)                            # [2048, 2048]
        gEE, gEO = _fold_cols(gE)                          # [2048, 1024]
        g3P, g3M = _fold_cols(gEE)                         # [2048, 512]
        xr = np.ascontiguousarray(
            np.concatenate([g3P, g3M, gEO], axis=1).astype(bf16))
        xq = np.ascontiguousarray(gO.astype(bf16))
        ins.append((xr, xq))

    in_maps = []
    for k in range(N_CORES):
        par = 0 if k < 4 else 1
        ksh = k % 4
        cm = cmt_e if par == 0 else cmt_o
        xr, xq = ins[par]
        in_maps.append({
            "xr": xr,
            "xq": xq,
            "cmt": np.ascontiguousarray(cm[:, ksh * TS:(ksh + 1) * TS]),
            "cne": cne,
            "ceo": ceo,
            "cnto": cnto,
        })
    res = run_bass_kernel_spmd(nc, in_maps, core_ids=list(range(N_CORES)),
                               trace=trace)

    out = np.empty((M, N), dtype=np.float32)
    tmp = np.empty((TS, N), dtype=np.float32)
    for k in range(N_CORES):
        r = np.asarray(res.results[k]["out"], dtype=np.float32)
        par = 0 if k < 4 else 1
        t0 = (k % 4) * TS
        rows = slice(2 * t0 + par, 2 * (t0 + TS) + par, 2)
        tmp[:, vmap] = r
        out[rows] = tmp
    return out, res.exec_time_ns, res


def kernel(x):
    out, _, _ = _run_res(np.asarray(x), trace=False)
    return out
